# revision 46
# baseline (speedup 1.0000x reference)
"""Trainium2 Bass/Tile kernel for the DNC (scatter_memory) problem.

Strategy: pure data parallel. B=32 samples are split 4-per-core across 8
NeuronCores. Each core runs the full T=256 recurrence with every piece of
DNC state resident in SBUF:

  - controller state  h, c            in "B-layout"   [4, 64]
  - memory M, usage, link, precedence in "BN-layout"  [128(b*32+n), *]
  - read weights rw                   in BN-layout    [128, 4(heads)]

Bridges between layouts:
  - B-layout -> BN-layout broadcast: one PE matmul with a constant selection
    matrix S4 [4,128] (out[(b,n), f] = rhs[b, f]).
  - BN-scalar [128,1] -> per-sample row vector [128,32]: broadcast along
    free then a DVE 32x32 block transpose (each sample occupies exactly one
    32-partition block).
  - softmax over memory cells (partition dim!) is done by transposing into
    rows, doing a free-dim softmax, and transposing back.

The controller matmuls run with activations transposed ([dim, batch] lhsT)
so weights stay as plain SBUF-resident rhs operands; biases are folded in
via constant-one rows in the lhsT. w_out/w_fc are folded on the host into a
single [16, 320] output projection (pure weight preprocessing).

All matmuls run in plain fp32 (exact; 4 cycles/row). Activations are
exp/ln-based so the ACT engine never swaps its LUT table set.
"""

import os
import numpy as np
from contextlib import ExitStack

import concourse.bass as bass
import concourse.bacc as bacc
import concourse.mybir as mybir
import concourse.tile as tile
from concourse.bass_utils import run_bass_kernel_spmd

FP = mybir.dt.float32
FR = mybir.dt.float32r
AF = mybir.ActivationFunctionType
OP = mybir.AluOpType
AX = mybir.AxisListType

MS, H, N, V, W = 64, 4, 32, 16, 64
B = 32
T_FULL = 256
NCORES = 8
BL = B // NCORES            # 4 samples per core
P = BL * N                  # 128 partitions
IF = 799
EPS = 1e-6

# ---- interface vector offsets (jnp.split order in the reference) ----
_sizes = [H * W, H, W, 1, W, W, H, 1, 1, 3 * H, H * W, W, 2 * H]
_names = ["rkeys", "rbeta", "wkey", "wbeta", "erase", "wvec", "free",
          "ag", "wg", "modes", "rmask", "wmask", "sharp"]
IOF = {}
_o = 0
for _nm, _s in zip(_names, _sizes):
    IOF[_nm] = (_o, _o + _s)
    _o += _s
assert _o == IF

# ---- staging buffer layout (rhs of the S4 broadcast matmul) ----
SG = {"rk2": (0, 256), "rmsq": (256, 512), "wmsq": (512, 576),
      "rnmk": (576, 580), "rbeta": (580, 584), "wbeta": (584, 585),
      "sharp": (585, 593), "wnmk": (593, 594), "free": (594, 598),
      "ag": (598, 599), "wg": (599, 600), "modes": (600, 612),
      "wk2": (612, 676), "erase": (676, 740), "wvec": (740, 804)}
SGW = 804  # staging width


def _emit(ctx, tc, T, io):
    nc = tc.nc
    V_, S_, G_ = nc.vector, nc.scalar, nc.gpsimd

    def mm(out, lhsT, rhs, start=True, stop=True):
        nc.tensor.matmul(out, lhsT, rhs,
                         start=start, stop=stop, skip_group_check=True)

    # sigmoid/tanh via exp so the ACT engine stays on one LUT set
    # (natural_log_exp_and_others) -- a LoadActFuncSet table swap costs
    # ~1.3us and the scheduler interleaves functions freely.
    def sigmoid_(wk_pool, dst, src, tag):
        e = wk_pool.tile([src.shape[0], src.free_size()], FP, tag=tag,
                         name=tag)
        nc.scalar.activation(e, src, AF.Exp, scale=-1.0)
        nc.vector.tensor_scalar(e, e, 1.0, None, OP.add)
        nc.vector.reciprocal(dst, e)

    def tanh_(wk_pool, dst, src, tag):
        e = wk_pool.tile([src.shape[0], src.free_size()], FP, tag=tag,
                         name=tag)
        nc.scalar.activation(e, src, AF.Exp, scale=-2.0)
        nc.vector.tensor_scalar(e, e, 1.0, None, OP.add)
        nc.vector.reciprocal(e, e)
        nc.vector.tensor_scalar(dst, e, 2.0, -1.0, OP.mult, OP.add)

    cons = ctx.enter_context(tc.tile_pool(name="cons", bufs=1))
    st = ctx.enter_context(tc.tile_pool(name="st", bufs=1))
    wk = ctx.enter_context(tc.tile_pool(name="wk", bufs=3))
    psA = ctx.enter_context(tc.tile_pool(name="psA", bufs=1, space="PSUM"))
    psB = ctx.enter_context(tc.tile_pool(name="psB", bufs=1, space="PSUM"))
    psS = ctx.enter_context(tc.tile_pool(name="psS", bufs=4, space="PSUM"))

    def ps(shape):
        return psS.tile(shape, FP, tag="ps", name="ps")

    # ---------------- constants -> SBUF ----------------
    def load(name, shape):
        t = cons.tile(list(shape), FP, tag=name)
        nc.sync.dma_start(out=t, in_=io[name])
        return t

    emb_sb = load("emb", (V, MS))
    wcat1 = load("wcat1", (128, 4 * MS))
    cb1 = load("cb1", (BL, 4 * MS))      # exp(-b_lstm)
    cb2 = load("cb2", (BL, MS))          # exp(-2*b_lstm[g_ slice])
    wifb = load("wifb", (65, IF))
    weffh = load("weffh", (65, V))
    s4c = load("s4c", (BL, P))
    tric = cons.tile([P, N], mybir.dt.int32, tag="tric")
    nc.sync.dma_start(out=tric, in_=io["tric"])
    ndiagc = load("ndiagc", (P, N))
    dshc = load("dshc", (P, 2 * H))
    iotac = load("iotac", (V, 1))
    identc = load("identc", (BL, BL))

    # w_ih.T rows for the read-vector part: head pairs stacked to K=128
    wr = io["wreads"]
    wra = cons.tile([128, 4 * MS], FP, tag="wra")
    nc.sync.dma_start(out=wra, in_=wr[0:128, :])
    wrb = cons.tile([128, 4 * MS], FP, tag="wrb")
    nc.sync.dma_start(out=wrb, in_=wr[128:256, :])
    # w_eff.T read-part rows as [128, 2V]: col-block p holds head pair
    # (2p, 2p+1) stacked on partitions (matches readsTa/readsTb layout)
    weffr = cons.tile([128, 2 * V], FP, tag="weffr")
    we = io["weffr"]
    nc.sync.dma_start(out=weffr[0:64, 0:V], in_=we[0:64, :])
    nc.sync.dma_start(out=weffr[64:128, 0:V], in_=we[64:128, :])
    nc.sync.dma_start(out=weffr[0:64, V:2 * V], in_=we[128:192, :])
    nc.sync.dma_start(out=weffr[64:128, V:2 * V], in_=we[192:256, :])

    eps_t = cons.tile([P, 1], FP, tag="eps_t")
    V_.memset(eps_t, EPS)
    ones_c = cons.tile([P, N], FP, tag="ones_c")
    V_.memset(ones_c, 1.0)

    # ---------------- persistent state ----------------
    def zstate(name, shape):
        t = st.tile(list(shape), FP, tag=name)
        V_.memset(t, 0.0)
        return t

    hTb = zstate("hTb", (65, BL))        # rows 0:64 hT, row 64 = ones
    V_.memset(hTb[64:65, :], 1.0)
    ciT = zstate("ciT", (P, BL))         # rows 0:64 xtT, 64:128 hT
    c4 = zstate("c4", (BL, MS))
    M = zstate("M", (P, W))
    M2 = zstate("M2", (P, W))
    usage = zstate("usage", (P, 1))
    prec = zstate("prec", (P, 1))
    link = zstate("link", (P, N))
    linkT = zstate("linkT", (P, N))
    rw = zstate("rw", (P, H))
    ww = zstate("ww", (P, 1))
    # wide block-diag rw: bdwf has rw[b,h,:] in column b*32+h, bdwb in
    # column b*32+4+h. Used as PE lhsT so fw/bw land on spread-out rows.
    bdwf = zstate("bdwf", (P, P))
    bdwb = zstate("bdwb", (P, P))
    readsTa = zstate("readsTa", (P, BL))   # rows 0:64 head0, 64:128 head1
    readsTb = zstate("readsTb", (P, BL))   # rows 0:64 head2, 64:128 head3
    spad = zstate("spad", (P, N))        # read-score pad (cols 4:32 stay 0)
    Xt = st.tile([MS, BL * T], FP, tag="Xt")
    L = st.tile([BL, V, T], FP, tag="L")

    # ---------------- embedding: Xt[d, b*T+t] = emb[tok[b,t], d] ----------
    tokb = wk.tile([V, BL * T], FP, tag="tokb")
    nc.sync.dma_start(out=tokb, in_=io["tok"].to_broadcast((V, BL * T)))
    oh = wk.tile([V, BL * T], FP, tag="oh")
    V_.tensor_scalar(oh, tokb, iotac, None, OP.is_equal)
    for c0 in range(0, BL * T, 512):
        c1 = min(c0 + 512, BL * T)
        xt_ps = ps([MS, c1 - c0])
        mm(xt_ps, emb_sb, oh[:, c0:c1])
        S_.copy(Xt[:, c0:c1], xt_ps)

    # ---------------- recurrence ----------------
    for t in range(T):
        # ---- A: LSTM controller ----
        S_.copy(ciT[0:64, :], Xt[:, t::T])
        g_ps = ps([BL, 4 * MS])
        mm(g_ps, ciT, wcat1, start=True, stop=False)
        mm(g_ps, readsTa, wra, start=False, stop=False)
        mm(g_ps, readsTb, wrb, start=False, stop=True)
        # sigma(g+b) = 1/(1+exp(-g)*exp(-b)); exp(-b), exp(-2b) are consts
        sio = wk.tile([BL, 4 * MS], FP, tag="sio")
        e_sio = wk.tile([BL, 4 * MS], FP, tag="e_sio")
        S_.activation(e_sio, g_ps, AF.Exp, scale=-1.0)
        V_.scalar_tensor_tensor(e_sio, e_sio, 1.0, cb1, OP.mult, OP.mult)
        V_.tensor_scalar(e_sio, e_sio, 1.0, None, OP.add)
        V_.reciprocal(sio, e_sio)
        tg = wk.tile([BL, MS], FP, tag="tg")
        e_tg = wk.tile([BL, MS], FP, tag="e_tg")
        S_.activation(e_tg, g_ps[:, 128:192], AF.Exp, scale=-2.0)
        V_.scalar_tensor_tensor(e_tg, e_tg, 1.0, cb2, OP.mult, OP.mult)
        V_.tensor_scalar(e_tg, e_tg, 1.0, None, OP.add)
        V_.reciprocal(e_tg, e_tg)
        V_.tensor_scalar(tg, e_tg, 2.0, -1.0, OP.mult, OP.add)
        it = wk.tile([BL, MS], FP, tag="it")
        V_.tensor_tensor(it, sio[:, 0:64], tg, OP.mult)
        V_.tensor_tensor(c4, sio[:, 64:128], c4, OP.mult)
        V_.tensor_tensor(c4, c4, it, OP.add)
        tc_ = wk.tile([BL, MS], FP, tag="tc_")
        tanh_(wk, tc_, c4, "e_tc")
        h4 = wk.tile([BL, MS], FP, tag="h4")
        V_.tensor_tensor(h4, sio[:, 192:256], tc_, OP.mult)
        hT_ps = ps([MS, BL])
        nc.tensor.transpose(hT_ps, h4, identc)
        V_.tensor_copy(hTb[0:64, :], hT_ps)
        S_.copy(ciT[64:128, :], hT_ps)

        # ---- interface vector ----
        if_ps = psA.tile([BL, 1024], FP, tag="iface")
        mm(if_ps[:, 0:512], hTb, wifb[:, 0:512])
        mm(if_ps[:, 512:IF], hTb, wifb[:, 512:IF])

        def ifc(nm):
            a, b = IOF[nm]
            return if_ps[:, a:b]

        # ---- B: interface processing -> stg ----
        stg = wk.tile([BL, SGW], FP, tag="stg")

        def sg(nm):
            a, b = SG[nm]
            return stg[:, a:b]

        # sigma(x)^2 = 1/(1+exp(-x))^2 for rmask||wmask (contiguous)
        emsk = wk.tile([BL, H * W + W], FP, tag="emsk")
        S_.activation(emsk, if_ps[:, IOF["rmask"][0]:IOF["wmask"][1]],
                      AF.Exp, scale=-1.0)
        V_.tensor_scalar(emsk, emsk, 1.0, None, OP.add)
        V_.tensor_tensor(emsk, emsk, emsk, OP.mult)
        V_.reciprocal(stg[:, SG["rmsq"][0]:SG["wmsq"][1]], emsk)
        V_.tensor_tensor(sg("rk2"), ifc("rkeys"), sg("rmsq"), OP.mult)
        V_.tensor_tensor(sg("wk2"), ifc("wkey"), sg("wmsq"), OP.mult)
        # squared key norms (sqrt is folded into the exp(0.5*ln(.)) below)
        tr_ = wk.tile([BL, H * W], FP, tag="tr_")
        V_.tensor_tensor(tr_, ifc("rkeys"), sg("rk2"), OP.mult)
        V_.tensor_reduce(sg("rnmk"), tr_.rearrange("p (h w) -> p h w", h=H),
                         AX.X, OP.add)
        tw_ = wk.tile([BL, W], FP, tag="tw_")
        V_.tensor_tensor(tw_, ifc("wkey"), sg("wk2"), OP.mult)
        V_.tensor_reduce(sg("wnmk"), tw_, AX.X, OP.add)
        def softplus(dst, src, tmp):
            # softplus(x) = relu(x) + ln(1 + exp(-|x|))
            S_.activation(tmp, src, AF.Abs)
            S_.activation(tmp, tmp, AF.Exp, scale=-1.0)
            S_.activation(tmp, tmp, AF.Ln, bias=1.0)
            S_.activation(dst, src, AF.Relu)
            V_.tensor_tensor(dst, dst, tmp, OP.add)

        bsh = stg[:, SG["rbeta"][0]:SG["sharp"][1]]
        S_.copy(sg("rbeta"), ifc("rbeta"))
        S_.copy(sg("wbeta"), ifc("wbeta"))
        S_.copy(sg("sharp"), ifc("sharp"))
        spt = wk.tile([BL, 13], FP, tag="spt")
        softplus(bsh, bsh, spt)
        V_.tensor_scalar(bsh, bsh, 1.0, None, OP.add)
        sigmoid_(wk, stg[:, SG["free"][0]:SG["wg"][1]],
                 if_ps[:, IOF["free"][0]:IOF["wg"][1]], "e_fg")
        # modes softmax over triples (inputs are O(1): no max-subtract)
        ma = wk.tile([BL, 3 * H], FP, tag="ma")
        mav = ma.rearrange("p (h k) -> p h k", h=H)
        S_.activation(ma, ifc("modes"), AF.Exp)
        ms4 = wk.tile([BL, H], FP, tag="ms4")
        V_.tensor_reduce(ms4, mav, AX.X, OP.add)
        V_.reciprocal(ms4, ms4)
        V_.tensor_tensor(sg("modes").rearrange("p (h k) -> p h k", h=H),
                         mav, ms4.to_broadcast((BL, H, 3)), OP.mult)
        sigmoid_(wk, sg("erase"), ifc("erase"), "e_er")
        V_.tensor_copy(sg("wvec"), ifc("wvec"))

        # ---- C: broadcasts ----
        bc = psB.tile([P, SGW], FP, tag="bc")
        mm(bc[:, 0:512], s4c, stg[:, 0:512])         # ready first, PE warm
        mm(bc[:, 512:SGW], s4c, stg[:, 512:SGW])

        def bcs(nm):
            a, b = SG[nm]
            return bc[:, a:b]

        # ---- D: retention / usage / allocation / write addressing ----
        frw = wk.tile([P, H], FP, tag="frw")
        V_.tensor_tensor(frw, bcs("free"), rw, OP.mult)
        G_.tensor_scalar(frw, frw, -1.0, 1.0, OP.mult, OP.add)
        p2 = wk.tile([P, 2], FP, tag="p2")
        V_.tensor_tensor(p2, frw[:, 0:2], frw[:, 2:4], OP.mult)
        psi = wk.tile([P, 1], FP, tag="psi")
        V_.tensor_tensor(psi, p2[:, 0:1], p2[:, 1:2], OP.mult)
        # usage = (usage + ww - usage*ww) * psi   (ww = previous step's)
        omw = wk.tile([P, 1], FP, tag="omw")
        G_.tensor_scalar(omw, ww, -1.0, 1.0, OP.mult, OP.add)
        V_.scalar_tensor_tensor(usage, usage, omw, ww, OP.mult, OP.add)
        V_.tensor_tensor(usage, usage, psi, OP.mult)
        # u row-broadcast via block transpose of a broadcast-read AP
        ub = wk.tile([P, N], FP, tag="ub")
        V_.transpose(ub, usage.to_broadcast((P, N)))
        # C[i,j] = (u_j < u_i) | (u_j == u_i & j < i)
        ltm = wk.tile([P, N], mybir.dt.int32, tag="ltm")
        V_.tensor_scalar(ltm, ub, usage, None, OP.is_lt)
        lem = wk.tile([P, N], mybir.dt.int32, tag="lem")
        V_.tensor_scalar(lem, ub, usage, None, OP.is_le)
        cm = wk.tile([P, N], mybir.dt.int32, tag="cm")
        V_.select(cm, tric, lem, ltm)
        # factors: F = 1 - C*(1-u_j) = select(C, u_j, 1)
        s1 = wk.tile([P, N], FP, tag="s1")
        V_.select(s1, cm, ub, ones_c)
        q16 = wk.tile([P, 16], FP, tag="q16")
        V_.tensor_tensor(q16, s1[:, 0:16], s1[:, 16:32], OP.mult)
        q8 = wk.tile([P, 8], FP, tag="q8")
        V_.tensor_tensor(q8, q16[:, 0:8], q16[:, 8:16], OP.mult)
        q4 = wk.tile([P, 4], FP, tag="q4")
        V_.tensor_tensor(q4, q8[:, 0:4], q8[:, 4:8], OP.mult)
        q2 = wk.tile([P, 2], FP, tag="q2")
        V_.tensor_tensor(q2, q4[:, 0:2], q4[:, 2:4], OP.mult)
        alloc = wk.tile([P, 1], FP, tag="alloc")
        V_.tensor_tensor(alloc, q2[:, 0:1], q2[:, 1:2], OP.mult)
        omu = wk.tile([P, 1], FP, tag="omu")
        G_.tensor_scalar(omu, usage, -1.0, 1.0, OP.mult, OP.add)
        V_.tensor_tensor(alloc, alloc, omu, OP.mult)
        # write content addressing on OLD M
        jk1 = wk.tile([P, W], FP, tag="jk1")
        dw = wk.tile([P, 1], FP, tag="dw")
        V_.scalar_tensor_tensor(jk1, M, 1.0, bcs("wk2"), OP.mult, OP.mult,
                                accum_out=dw)
        jk2 = wk.tile([P, W], FP, tag="jk2")
        nw = wk.tile([P, 1], FP, tag="nw")
        V_.scalar_tensor_tensor(jk2, M2, 1.0, bcs("wmsq"), OP.mult, OP.mult,
                                accum_out=nw)
        # den = sqrt(nmm2_w * wnmk2) + EPS, sqrt via exp(0.5*ln(.))
        den = wk.tile([P, 1], FP, tag="den")
        V_.tensor_scalar(den, nw, bcs("wnmk"), 1e-30, OP.mult, OP.max)
        S_.activation(den, den, AF.Ln)
        S_.activation(den, den, AF.Exp, scale=0.5)
        V_.tensor_scalar(den, den, EPS, None, OP.add)
        V_.reciprocal(den, den)
        swv = wk.tile([P, 1], FP, tag="swv")
        V_.scalar_tensor_tensor(swv, dw, den, bcs("wbeta"), OP.mult, OP.mult)
        # softmax over cells (partition dim) via double transpose
        swt = wk.tile([P, N], FP, tag="swt")
        V_.transpose(swt, swv.to_broadcast((P, N)))
        S_.activation(swt, swt, AF.Exp)
        smw = wk.tile([P, 1], FP, tag="smw")
        V_.tensor_reduce(smw, swt, AX.X, OP.add)
        V_.reciprocal(smw, smw)
        cwt = wk.tile([P, N], FP, tag="cwt")
        V_.transpose(cwt, swt)
        # every transposed row of a sample is the same score vector, so
        # normalize just the extracted column; fold (1-ag) into the same op
        omag = wk.tile([P, 1], FP, tag="omag")
        V_.tensor_scalar(omag, bcs("ag"), -1.0, 1.0, OP.mult, OP.add)
        t2_ = wk.tile([P, 1], FP, tag="t2_")
        V_.tensor_scalar(t2_, cwt[:, 0:1], smw, omag, OP.mult, OP.mult)
        t3_ = wk.tile([P, 1], FP, tag="t3_")
        V_.scalar_tensor_tensor(t3_, alloc, bcs("ag"), t2_, OP.mult, OP.add)
        V_.tensor_scalar(ww, t3_, bcs("wg"), None, OP.mult)

        # ---- E: memory write; M = M*psi*(1 - ww*erase) + ww*wvec ----
        e1 = wk.tile([P, W], FP, tag="e1")
        V_.tensor_scalar(e1, bcs("erase"), ww, -1.0, OP.mult, OP.mult)
        V_.tensor_scalar(e1, e1, 1.0, psi, OP.add, OP.mult)
        wv1 = wk.tile([P, W], FP, tag="wv1")
        V_.tensor_scalar(wv1, bcs("wvec"), ww, None, OP.mult)
        V_.tensor_tensor(M, M, e1, OP.mult)
        V_.tensor_tensor(M, M, wv1, OP.add)
        V_.tensor_tensor(M2, M, M, OP.mult)
        # link updates need ww and prec as per-sample row vectors
        wwb = wk.tile([P, N], FP, tag="wwb")
        V_.transpose(wwb, ww.to_broadcast((P, N)))
        prb = wk.tile([P, N], FP, tag="prb")
        V_.transpose(prb, prec.to_broadcast((P, N)))
        omwi = wk.tile([P, 1], FP, tag="omwi")
        G_.tensor_scalar(omwi, ww, -1.0, 1.0, OP.mult, OP.add)
        f_ = wk.tile([P, N], FP, tag="f_")
        V_.tensor_scalar(f_, wwb, -1.0, omwi, OP.mult, OP.add)
        G_.tensor_tensor(link, f_, link, OP.mult)
        tl1 = wk.tile([P, N], FP, tag="tl1")
        V_.tensor_scalar(tl1, prb, ww, None, OP.mult)
        G_.tensor_tensor(link, link, tl1, OP.add)
        G_.tensor_tensor(link, link, ndiagc, OP.mult)
        G_.tensor_tensor(linkT, f_, linkT, OP.mult)
        tl2 = wk.tile([P, N], FP, tag="tl2")
        V_.tensor_scalar(tl2, wwb, prec, None, OP.mult)
        G_.tensor_tensor(linkT, linkT, tl2, OP.add)
        G_.tensor_tensor(linkT, linkT, ndiagc, OP.mult)
        # prec = (1 - sum(ww)) * prec + ww
        sww = wk.tile([P, 1], FP, tag="sww")
        V_.tensor_reduce(sww, wwb, AX.X, OP.add)
        G_.tensor_scalar(sww, sww, -1.0, 1.0, OP.mult, OP.add)
        V_.scalar_tensor_tensor(prec, prec, sww, ww, OP.mult, OP.add)

        # ---- F: forward/backward + read addressing ----
        # block-diag of OLD rw into the wide tiles
        for b in range(BL):
            G_.tensor_copy(bdwf[N * b:N * (b + 1), N * b:N * b + H],
                           rw[N * b:N * (b + 1), :])
            G_.tensor_copy(bdwb[N * b:N * (b + 1), N * b + H:N * b + 2 * H],
                           rw[N * b:N * (b + 1), :])
        # rows b*32+h = fw[b,h,:], rows b*32+4+h = bw[b,h,:], rest zero
        pad_ps = ps([P, N])
        mm(pad_ps, bdwf, linkT, start=True, stop=False)
        mm(pad_ps, bdwb, link, start=False, stop=True)
        fbs = wk.tile([P, N], FP, tag="fbs")
        S_.activation(fbs, pad_ps, AF.Ln, bias=eps_t)
        dsh_t = wk.tile([P, 2 * H], FP, tag="dsh_t")
        V_.tensor_tensor(dsh_t, bcs("sharp"), dshc, OP.mult)
        sharp_pp = wk.tile([P, 1], FP, tag="sharp_pp")
        V_.tensor_reduce(sharp_pp, dsh_t, AX.X, OP.add)
        S_.activation(fbs, fbs, AF.Exp, scale=sharp_pp)
        fs_ = wk.tile([P, 1], FP, tag="fs_")
        V_.tensor_reduce(fs_, fbs, AX.X, OP.add)
        V_.reciprocal(fs_, fs_)
        V_.tensor_scalar(fbs, fbs, fs_, None, OP.mult)
        fbt = wk.tile([P, N], FP, tag="fbt")
        V_.transpose(fbt, fbs)                     # cols 0:4 fw_bn, 4:8 bw_bn
        # read content scores (NEW M)
        dr = spad[:, 0:H]
        nr = wk.tile([P, H], FP, tag="nr")
        m_bc = bass.AP(tensor=M.tensor, offset=M.offset,
                       ap=[M.ap[0], [0, H], M.ap[1]])
        m2_bc = bass.AP(tensor=M2.tensor, offset=M2.offset,
                        ap=[M2.ap[0], [0, H], M2.ap[1]])
        jka = wk.tile([P, H, W], FP, tag="jka")
        V_.tensor_tensor(jka, m_bc,
                         bc[:, 0:256].rearrange("p (h w) -> p h w", h=H),
                         OP.mult)
        V_.tensor_reduce(dr, jka, AX.X, OP.add)
        jkb = wk.tile([P, H, W], FP, tag="jkb")
        V_.tensor_tensor(jkb, m2_bc,
                         bc[:, 256:512].rearrange("p (h w) -> p h w", h=H),
                         OP.mult)
        V_.tensor_reduce(nr, jkb, AX.X, OP.add)
        # 1 / (sqrt(nmm2 * rnmk2) + EPS), sqrt via exp(0.5*ln(.))
        V_.tensor_tensor(nr, nr, bcs("rnmk"), OP.mult)
        V_.tensor_scalar(nr, nr, 1e-30, None, OP.max)
        S_.activation(nr, nr, AF.Ln)
        S_.activation(nr, nr, AF.Exp, scale=0.5)
        V_.tensor_scalar(nr, nr, EPS, None, OP.add)
        V_.reciprocal(nr, nr)
        V_.tensor_tensor(dr, dr, nr, OP.mult)
        V_.tensor_tensor(dr, dr, bcs("rbeta"), OP.mult)
        # read softmax over cells via double transpose
        st_ = wk.tile([P, N], FP, tag="st_")
        V_.transpose(st_, spad)
        S_.activation(st_, st_, AF.Exp)
        smr = wk.tile([P, 1], FP, tag="smr")
        V_.tensor_reduce(smr, st_, AX.X, OP.add)
        V_.reciprocal(smr, smr)
        V_.tensor_scalar(st_, st_, smr, None, OP.mult)
        crt = wk.tile([P, N], FP, tag="crt")
        V_.transpose(crt, st_)                     # cols 0:4 = cr_bn
        # rw = m0*bw + m1*cr + m2*fw
        mo = SG["modes"][0]
        m0 = bc[:, mo + 0:mo + 10:3]
        m1 = bc[:, mo + 1:mo + 11:3]
        m2 = bc[:, mo + 2:mo + 12:3]
        nw1 = wk.tile([P, H], FP, tag="nw1")
        V_.tensor_tensor(nw1, fbt[:, H:2 * H], m0, OP.mult)
        nw2 = wk.tile([P, H], FP, tag="nw2")
        V_.tensor_tensor(nw2, crt[:, 0:H], m1, OP.mult)
        V_.tensor_tensor(nw1, nw1, nw2, OP.add)
        nw3 = wk.tile([P, H], FP, tag="nw3")
        V_.tensor_tensor(nw3, fbt[:, 0:H], m2, OP.mult)
        V_.tensor_tensor(rw, nw1, nw3, OP.add)
        # block-diag of NEW rw, then reads (transposed): [w, (b,h)]
        for b in range(BL):
            G_.tensor_copy(bdwf[N * b:N * (b + 1), N * b:N * b + H],
                           rw[N * b:N * (b + 1), :])
        bdview = bdwf.rearrange("p (b c) -> p b c", b=BL)[:, :, 0:H]
        rds_ps = ps([W, BL * H])
        mm(rds_ps, M, bdview)
        V_.tensor_copy(readsTa[0:64, :], rds_ps[:, 0::H])
        S_.copy(readsTa[64:128, :], rds_ps[:, 1::H])
        V_.tensor_copy(readsTb[0:64, :], rds_ps[:, 2::H])
        S_.copy(readsTb[64:128, :], rds_ps[:, 3::H])

        # ---- H: output logits for this step ----
        lg_ps = ps([BL, V])
        mm(lg_ps, hTb, weffh, start=True, stop=False)
        mm(lg_ps, readsTa, weffr[:, 0:V], start=False, stop=False)
        mm(lg_ps, readsTb, weffr[:, V:2 * V], start=False, stop=True)
        S_.copy(L[:, :, t], lg_ps)

    nc.sync.dma_start(out=io["out"], in_=L)


_ACT_TABLES_PATCHED = False


def _patch_act_tables():
    """Make every activation resolve to natural_log_exp_and_others.

    Bacc's insert_act_table_loads picks the FIRST act-func-set containing
    each function, so alternating Exp/Ln costs a ~1.3us table reload each
    time even though one set holds both. This kernel only uses functions
    from natural_log_exp_and_others (exp, ln, copy, identity, square, abs,
    relu), so blank out all other sets -> exactly one table load total.
    """
    global _ACT_TABLES_PATCHED
    if _ACT_TABLES_PATCHED:
        return
    import concourse.hw_specs as hw_specs
    import concourse.bacc as bacc_mod
    orig = hw_specs.get_activation_tables
    keep = "natural_log_exp_and_others"

    def patched(module_arch):
        tabs = orig(module_arch)
        assert keep in tabs, tabs.keys()
        return {k: (v if k == keep else set()) for k, v in tabs.items()}

    hw_specs.get_activation_tables = patched
    bacc_mod.get_activation_tables = patched
    _ACT_TABLES_PATCHED = True


def build_program(T, compile=True):
    _patch_act_tables()
    nc = bacc.Bacc("TRN2", target_bir_lowering=False, debug=False)
    io = {}

    def din(name, shape):
        io[name] = nc.dram_tensor(name, list(shape), FP,
                                  kind="ExternalInput").ap()

    din("tok", (1, BL * T))
    din("emb", (V, MS))
    din("wcat1", (128, 4 * MS))
    din("wreads", (256, 4 * MS))
    din("cb1", (BL, 4 * MS))
    din("cb2", (BL, MS))
    din("wifb", (65, IF))
    din("weffh", (65, V))
    din("weffr", (256, V))
    din("s4c", (BL, P))
    io["tric"] = nc.dram_tensor("tric", [P, N], mybir.dt.int32,
                                kind="ExternalInput").ap()
    din("ndiagc", (P, N))
    din("dshc", (P, 2 * H))
    din("iotac", (V, 1))
    din("identc", (BL, BL))
    io["out"] = nc.dram_tensor("out", [BL, V, T], FP,
                               kind="ExternalOutput").ap()

    with tile.TileContext(nc) as tctx, ExitStack() as ctx:
        _emit(ctx, tctx, T, io)
    if compile:
        nc.compile()
    return nc


def host_constants(emb, w_ih, w_hh, b_lstm, w_if, b_if, w_out, b_out,
                   w_fc, b_fc):
    """Preprocess weights (pure repacking + w_out/w_fc fold) -> const arrays."""
    f64 = np.float64
    w_ihT = np.ascontiguousarray(w_ih.T.astype(np.float32))    # [320, 256]
    w_hhT = np.ascontiguousarray(w_hh.T.astype(np.float32))    # [64, 256]
    wcat1 = np.concatenate([w_ihT[0:64], w_hhT], axis=0)       # [128, 256]
    wreads = np.ascontiguousarray(w_ihT[64:320])               # [256, 256]
    cb1 = np.tile(np.exp(-b_lstm.astype(np.float64))[None, :],
                  (BL, 1)).astype(np.float32)                    # [4, 256]
    cb2 = np.tile(np.exp(-2.0 * b_lstm[128:192].astype(np.float64))[None, :],
                  (BL, 1)).astype(np.float32)                    # [4, 64]
    wifb = np.concatenate([w_if.T.astype(np.float32),
                           b_if[None, :].astype(np.float32)], axis=0)  # [65,799]
    w_eff = (w_fc.astype(f64) @ w_out.astype(f64)).astype(np.float32)  # [16,320]
    b_eff = (w_fc.astype(f64) @ b_out.astype(f64)
             + b_fc.astype(f64)).astype(np.float32)            # [16]
    w_effT = np.ascontiguousarray(w_eff.T)                     # [320, 16]
    weffh = np.concatenate([w_effT[0:64], b_eff[None, :]], axis=0)  # [65, 16]
    weffr = np.ascontiguousarray(w_effT[64:320])               # [256, 16]

    s4c = np.zeros((BL, P), np.float32)
    for b in range(BL):
        s4c[b, N * b:N * (b + 1)] = 1.0
    ii, jj = np.meshgrid(np.arange(N), np.arange(N), indexing="ij")
    tri1 = (jj < ii).astype(np.float32)                        # [32, 32]
    tric = np.tile(tri1, (BL, 1)).astype(np.int32)             # [128, 32]
    ndiagc = np.tile((ii != jj).astype(np.float32), (BL, 1))   # [128, 32]
    dshc = np.zeros((P, 2 * H), np.float32)
    for b in range(BL):
        for j in range(H):
            dshc[N * b + j, 2 * j] = 1.0          # fw sharp selector
            dshc[N * b + H + j, 2 * j + 1] = 1.0  # bw sharp selector
    iotac = np.arange(V, dtype=np.float32)[:, None]
    identc = np.eye(BL, dtype=np.float32)
    return dict(emb=np.ascontiguousarray(emb.astype(np.float32)),
                wcat1=wcat1, wreads=wreads, cb1=cb1, cb2=cb2, wifb=wifb,
                weffh=weffh, weffr=weffr, s4c=s4c, tric=tric,
                ndiagc=ndiagc, dshc=dshc, iotac=iotac, identc=identc)


_CACHE = {}


def _get_nc(T):
    if T not in _CACHE:
        _CACHE[T] = build_program(T)
    return _CACHE[T]


def make_in_maps(inputs, T=None, ncores=NCORES):
    tokens = np.asarray(inputs["tokens"])
    if T is None:
        T = tokens.shape[1]
    consts = host_constants(
        np.asarray(inputs["emb"]), np.asarray(inputs["w_ih"]),
        np.asarray(inputs["w_hh"]), np.asarray(inputs["b_lstm"]),
        np.asarray(inputs["w_if"]), np.asarray(inputs["b_if"]),
        np.asarray(inputs["w_out"]), np.asarray(inputs["b_out"]),
        np.asarray(inputs["w_fc"]), np.asarray(inputs["b_fc"]))
    in_maps = []
    for c in range(ncores):
        tok = tokens[BL * c:BL * (c + 1), :T].astype(np.float32)
        m = dict(consts)
        m["tok"] = np.ascontiguousarray(tok.reshape(1, BL * T))
        in_maps.append(m)
    return in_maps


class _Runner:
    """Cached jitted shard_map executor (mirrors bass2jax.run_bass_via_pjrt
    but keeps the compiled callable so repeat kernel() calls skip re-trace)."""

    def __init__(self, nc, n_cores):
        import jax
        from jax.sharding import Mesh, PartitionSpec
        from jax.experimental.shard_map import shard_map
        from concourse import bass2jax

        bass2jax.install_neuronx_cc_hook()
        self.nc = nc
        self.n_cores = n_cores
        part_name = (nc.partition_id_tensor.name
                     if nc.partition_id_tensor else None)
        in_names, out_names, out_avals = [], [], []
        for alloc in nc.m.functions[0].allocations:
            if not isinstance(alloc, mybir.MemoryLocationSet):
                continue
            name = alloc.memorylocations[0].name
            if alloc.kind == "ExternalInput":
                if name != part_name:
                    in_names.append(name)
            elif alloc.kind == "ExternalOutput":
                out_names.append(name)
                out_avals.append(jax.core.ShapedArray(
                    tuple(alloc.tensor_shape), mybir.dt.np(alloc.dtype)))
        self.in_names, self.out_names, self.out_avals = \
            in_names, out_names, out_avals
        n_params, n_outs = len(in_names), len(out_avals)
        all_names = list(in_names + out_names)
        if part_name is not None:
            all_names.append(part_name)
        all_names = tuple(all_names)

        def _body(*args):
            operands = list(args)
            if part_name is not None:
                operands.append(bass2jax.partition_id_tensor())
            outs = bass2jax._bass_exec_p.bind(
                *operands, out_avals=tuple(out_avals), in_names=all_names,
                out_names=tuple(out_names),
                lowering_input_output_aliases=(),
                sim_require_finite=True, sim_require_nnan=True, nc=nc)
            return tuple(outs)

        devices = jax.devices()[:n_cores]
        mesh = Mesh(np.asarray(devices), ("core",))
        specs = (PartitionSpec("core"),) * (n_params + n_outs)
        self.fn = jax.jit(shard_map(_body, mesh=mesh, in_specs=specs,
                                    out_specs=(PartitionSpec("core"),) * n_outs,
                                    check_rep=False), keep_unused=True)
        self.jax = jax

    def __call__(self, in_maps):
        np_ = np
        concat_in = [np_.concatenate([np_.asarray(m[name]) for m in in_maps],
                                     axis=0) for name in self.in_names]
        zeros = [np_.zeros((self.n_cores * a.shape[0], *a.shape[1:]), a.dtype)
                 for a in self.out_avals]
        outs = self.fn(*concat_in, *zeros)
        outs = [np_.asarray(o) for o in outs]
        return outs  # list over out_names of concatenated [n_cores*d0, ...]


_RUNNERS = {}


def _get_runner(T):
    if T not in _RUNNERS:
        _RUNNERS[T] = _Runner(_get_nc(T), NCORES)
    return _RUNNERS[T]


def kernel(**inputs):
    T = T_FULL
    in_maps = make_in_maps(inputs, T=T)
    out = _get_runner(T)(in_maps)[0]       # [NCORES*BL, V, T]
    return np.ascontiguousarray(out).astype(np.float32)
